# revision 34
# baseline (speedup 1.0000x reference)
"""Multi-head attention (B=2, S=4096, D=1024, H=16, HD=64) on 8 trn2 cores.

Sharding: core c -> batch b = c//4, head-group g = c%4 (4 heads per core).
Each core: Q/K/V projections for its heads on its batch, attention, and the
partial output ctx @ Wo[rows of its heads]. Host sums the 4 partials per
batch and adds bo.

Design notes (measured on HW, perfetto/NTFF traces):
  - The whole kernel is ActE(exp)/PE co-limited in phase 2; everything
    else is engine-balancing around those two walls.
  - All matmul operands bf16 (fp8 fails the 2e-2 gate: measured 4.1e-2
    end-to-end; bf16 measures ~7e-3). PSUM stays fp32.
  - exp: 5/8 of score tiles on the scalar engine (Exp activation,
    PSUM->SBUF bf16, 1/8 scale folded, no max-subtraction since scores
    are O(6)); 3/8 on the vector engine via a one-op int16-domain
    Schraudolph (bf16_bits = int16(x*2^7/ln2 + (127<<7) - 7), bitcast
    int16->bf16 is free). Softmax normalization cancels most of the
    Schraudolph error.
  - X^T via PE transposes in bf16 (cast on ActE first); 4 transposes
    batched per PSUM tile to amortize the DVE copy-out.
  - Q^T/K^T [dq, tok] bf16 with head-pairs packed on partition halves:
    scores = two 64-row PE tiles that the HW co-schedules (row groups).
    PV likewise runs as 64-row tile pairs into 4 separate accumulators —
    merging the k-halves into one accumulator (interleaved accumulation
    groups on one bank) crashes the exec unit, hence the final add.
  - V token-major bf16 with a ones column per head (V'[k, 65]); PV
    accumulates ctx^T and the softmax denominator together.
  - ctx'' stays resident in SBUF (bf16); phase 3 (normalize + Wo) is
    DMA-free until the output store.
  - PSUM is the scarce resource: phase 2 uses all 8 banks (2x2 score
    tiles + 4 accumulators), which is what prevents phase overlap.
"""

import os
from contextlib import ExitStack

import numpy as np

os.environ.setdefault("MYCRO_LOCAL_CACHE", "1")

import concourse.bass as bass
import concourse.tile as tile
from concourse import bacc, mybir
from concourse.bass_utils import run_bass_kernel_spmd
from concourse.masks import make_identity

F32 = mybir.dt.float32
F32R = mybir.dt.float32r
BF16 = mybir.dt.bfloat16
I16 = mybir.dt.int16
AF = mybir.ActivationFunctionType

S = 4096          # sequence length
D = 1024          # model dim
HC = 4            # heads per core
HD = 64           # head dim
DC = HC * HD      # 256 per-core projection width
NP = HC // 2      # head pairs per core
KT = S // 128     # 32 k-tiles
QC = S // 512     # 8 q-chunks of 512
TC = S // 512     # 8 tok-chunks of 512
SCALE = 1.0 / 8.0

# Schraudolph-exp constants (DVE offload), int16/bf16-bit domain:
# bf16_bits(exp(x)) ~ int16(x * 2^7/ln2 + (127<<7) - C), one DVE op with a
# free bitcast int16->bf16. The scores 1/8 scale is folded into A. The
# scalar engine saturates on exp; routing 3/8 of the tiles to the idle
# vector engine costs ~3% bounded relative error on those attention
# weights, washed out by the softmax normalization (end-to-end measured
# 6.5e-3 vs the 2e-2 gate).
EXP_A = float((1 << 7) / np.log(2.0) * SCALE)
EXP_B = float((127 << 7) - 7)
SCHRAUD_K = frozenset(k for k in range(KT) if k % 8 in (1, 3, 6))


def _emit(ctx: ExitStack, tc: tile.TileContext, ins: dict, out: bass.AP):
    nc = tc.nc
    X, Wq, bq, Wk, bk, Wv, bv, Wo = (
        ins["X"], ins["Wq"], ins["bq"], ins["Wk"], ins["bk"], ins["Wv"],
        ins["bv"], ins["Wo"],
    )

    const = ctx.enter_context(tc.tile_pool(name="const", bufs=1))
    ident = const.tile([128, 128], F32)
    make_identity(nc, ident[:])
    idb = const.tile([128, 128], BF16, tag="idb")
    nc.vector.tensor_copy(idb[:], ident[:])

    # All weights bf16 (X is cast to bf16 before the transpose/projections).
    wq_sb = const.tile([128, 8 * DC], BF16, tag="wq")
    wk_sb = const.tile([128, 8 * DC], BF16, tag="wk")
    wv_sb = const.tile([128, 8 * DC], BF16, tag="wv")
    wo_sb = const.tile([128, 2 * D], BF16, tag="wo")
    with tc.tile_pool(name="wstage", bufs=2) as wst:
        for dst, src, nchunks in ((wq_sb, Wq, 8), (wk_sb, Wk, 8),
                                  (wv_sb, Wv, 8), (wo_sb, Wo, 2)):
            stg = wst.tile([128, dst.shape[1]], F32, tag="wstg",
                           name=f"wstg_{src.name}")
            nc.gpsimd.dma_start(stg[:].rearrange("p (c d) -> p c d", c=nchunks),
                                src.rearrange("(c p) d -> p c d", p=128))
            nc.vector.tensor_copy(dst[:], stg[:])
    bq_sb = const.tile([128, 2], F32, tag="bq")
    bk_sb = const.tile([128, 2], F32, tag="bk")
    nc.gpsimd.dma_start(bq_sb[:], bq.rearrange("(c p) -> p c", p=128))
    nc.gpsimd.dma_start(bk_sb[:], bk.rearrange("(c p) -> p c", p=128))
    bv_bc = const.tile([128, DC], F32, tag="bv")
    nc.gpsimd.dma_start(bv_bc[:], bv.unsqueeze(0).to_broadcast([128, DC]))
    ones4 = const.tile([128, HC], F32, tag="ones4")
    nc.vector.memset(ones4[:], 1.0)

    # Activations live for the whole kernel (all bf16 now; ctx in SBUF).
    acts = ctx.enter_context(tc.tile_pool(name="acts", bufs=1))
    QT = [acts.tile([128, S], BF16, tag=f"qt{p}", name=f"qt{p}") for p in range(NP)]
    KT_ = [acts.tile([128, S], BF16, tag=f"kt{p}", name=f"ktile{p}") for p in range(NP)]
    VPA = acts.tile([128, KT, HC * 65], BF16, tag="vpa", name="vpa")
    VP = [VPA[:, k, :] for k in range(KT)]
    ctx_sb = acts.tile([65, HC, S], BF16, tag="ctxs", name="ctxs")

    # ---------------- Phase 1: X^T + projections ----------------
    with nc.named_scope("ph1"), \
         tc.tile_pool(name="xrow", bufs=8) as xrow, \
         tc.tile_pool(name="xb", bufs=8) as xbp, \
         tc.tile_pool(name="xt", bufs=16) as xtp, \
         tc.tile_pool(name="ps1", bufs=4, space="PSUM") as ps1, \
         tc.tile_pool(name="ps2", bufs=2, space="PSUM") as ps2:
        for t8 in range(TC):
            xts = [xrow.tile([128, D], F32, tag="xr", name=f"xr{t8}_{i}") for i in range(4)]
            xbs = [xbp.tile([128, D], BF16, tag="xb", name=f"xb{t8}_{i}") for i in range(4)]
            for tt in range(4):
                nc.sync.dma_start(xts[tt][:], X[t8 * 512 + tt * 128:
                                              t8 * 512 + (tt + 1) * 128, :])
                nc.scalar.copy(xbs[tt][:], xts[tt][:])
            xt = [xtp.tile([128, 512], BF16, tag="xt", name=f"xt{t8}_{i}") for i in range(8)]
            for dc in range(8):
                pt = ps1.tile([128, 4, 128], BF16, tag="tp")
                for tt in range(4):
                    nc.tensor.transpose(
                        pt[:, tt, :], xbs[tt][:, dc * 128:(dc + 1) * 128],
                        idb[:])
                nc.vector.tensor_copy(xt[dc][:], pt[:].rearrange("p a b -> p (a b)"))
            for p in range(NP):
                pq = ps2.tile([128, 512], F32, tag="pq")
                for dc in range(8):
                    nc.tensor.matmul(
                        pq[:], wq_sb[:, dc * DC + p * 128: dc * DC + (p + 1) * 128],
                        xt[dc][:], start=(dc == 0), stop=(dc == 7))
                nc.scalar.activation(
                    QT[p][:, t8 * 512:(t8 + 1) * 512], pq[:], AF.Identity,
                    bias=bq_sb[:, p:p + 1])
                pk = ps2.tile([128, 512], F32, tag="pq")
                for dc in range(8):
                    nc.tensor.matmul(
                        pk[:], wk_sb[:, dc * DC + p * 128: dc * DC + (p + 1) * 128],
                        xt[dc][:], start=(dc == 0), stop=(dc == 7))
                nc.vector.tensor_scalar_add(
                    KT_[p][:, t8 * 512:(t8 + 1) * 512], pk[:], bk_sb[:, p:p + 1])
            for tt in range(4):
                kt = t8 * 4 + tt
                pv = ps2.tile([128, 256], F32, tag="pv")
                for dc in range(8):
                    nc.tensor.matmul(
                        pv[:], xt[dc][:, tt * 128:(tt + 1) * 128],
                        wv_sb[:, dc * DC:(dc + 1) * DC],
                        start=(dc == 0), stop=(dc == 7))
                vdst = VP[kt][:].rearrange("p (h w) -> p h w", h=HC)[:, :, 0:64]
                nc.vector.scalar_tensor_tensor(
                    vdst, pv[:].rearrange("p (h w) -> p h w", h=HC), 1.0,
                    bv_bc[:].rearrange("p (h w) -> p h w", h=HC),
                    mybir.AluOpType.bypass, mybir.AluOpType.add)
                ones = VP[kt][:].rearrange("p (h w) -> p h w", h=HC)[:, :, 64:65]
                nc.vector.tensor_copy(ones, ones4[:].unsqueeze(2))

    # ---------------- Phase 2: attention ----------------
    with nc.named_scope("ph2"), \
         tc.tile_pool(name="sps", bufs=2, space="PSUM") as sps, \
         tc.tile_pool(name="pvs", bufs=4, space="PSUM") as pvs, \
         tc.tile_pool(name="et", bufs=8) as etp, \
         tc.tile_pool(name="eti", bufs=6) as etip:
        seq = [(p, qc, k) for p in range(NP) for qc in range(QC)
               for k in range(KT)]
        accs = {}
        ets = {}

        def s_step(i):
            p, qc, k = seq[i]
            qs = slice(qc * 512, (qc + 1) * 512)
            ks = slice(k * 128, (k + 1) * 128)
            st = sps.tile([128, 1024], F32, tag="st", name=f"st{p}_{qc}_{k}")
            nc.tensor.matmul(st[:, 0:512], KT_[p][0:64, ks],
                             QT[p][0:64, qs], start=True, stop=True)
            nc.tensor.matmul(st[:, 512:1024], KT_[p][64:128, ks],
                             QT[p][64:128, qs], start=True, stop=True)
            if k in SCHRAUD_K:
                ei = etip.tile([128, 1024], I16, tag="eti",
                               name=f"eti{p}_{qc}_{k}")
                nc.vector.tensor_scalar(ei[:], st[:], EXP_A, EXP_B,
                                        mybir.AluOpType.mult,
                                        mybir.AluOpType.add)
                et = ei[:].bitcast(BF16)
            else:
                ett = etp.tile([128, 1024], BF16, tag="et",
                               name=f"et{p}_{qc}_{k}")
                nc.scalar.activation(ett[:], st[:], AF.Exp, bias=0.0,
                                     scale=SCALE)
                et = ett[:]
            ets[i] = et

        LOOKAHEAD = 2
        for i in range(LOOKAHEAD):
            s_step(i)
        for i, (p, qc, k) in enumerate(seq):
            if i + LOOKAHEAD < len(seq):
                s_step(i + LOOKAHEAD)
            if k == 0:
                accs[(p, qc)] = [
                    pvs.tile([65, 512], F32, tag="acc", name=f"acc{p}_{qc}_{j2}")
                    for j2 in range(4)]
            acc = accs[(p, qc)]
            et = ets.pop(i)
            for j in range(2):
                h = 2 * p + j
                vs = slice(h * 65, (h + 1) * 65)
                es = slice(j * 512, (j + 1) * 512)
                nc.tensor.matmul(
                    acc[2 * j][:], VP[k][0:64, vs], et[0:64, es],
                    start=(k == 0), stop=(k == KT - 1),
                    skip_group_check=True)
                nc.tensor.matmul(
                    acc[2 * j + 1][:], VP[k][64:128, vs],
                    et[64:128, es],
                    start=(k == 0), stop=(k == KT - 1),
                    skip_group_check=True)
            if k == KT - 1:
                qs = slice(qc * 512, (qc + 1) * 512)
                for j in range(2):
                    h = 2 * p + j
                    btmp = etp.tile([65, 512], F32, tag="btmp",
                                    name=f"btmp{p}_{qc}_{j}")
                    nc.scalar.copy(btmp[:], acc[2 * j + 1][:])
                    nc.vector.tensor_add(ctx_sb[:, h, qs], acc[2 * j][:],
                                         btmp[:])
                del accs[(p, qc)]

    # ---------------- Phase 3: normalize + Wo ----------------
    with nc.named_scope("ph3"), \
         tc.tile_pool(name="ps3a", bufs=4, space="PSUM") as ps3a, \
         tc.tile_pool(name="ps3b", bufs=2, space="PSUM") as ps3b, \
         tc.tile_pool(name="po", bufs=2, space="PSUM") as pop, \
         tc.tile_pool(name="ctxn", bufs=4) as ctxnp, \
         tc.tile_pool(name="rcpp", bufs=12) as rcpp, \
         tc.tile_pool(name="ltp", bufs=6) as ltp, \
         tc.tile_pool(name="osb", bufs=4) as osbp:
        for t in range(S // 128):
            ts_ = slice(t * 128, (t + 1) * 128)
            lts = []
            for p in range(NP):
                ctxn = ctxnp.tile([128, 128], BF16, tag="ctxn",
                                  name=f"ctxn{t}_{p}")
                for j in range(2):
                    h = 2 * p + j
                    tp1 = ps3a.tile([128, 65], BF16, tag="tp1",
                                    name=f"tp1_{t}_{h}")
                    nc.tensor.transpose(tp1[:], ctx_sb[:, h, ts_],
                                        idb[0:65, 0:65])
                    rcp = rcpp.tile([128, 1], F32, tag="rcp",
                                    name=f"rcp{t}_{h}")
                    nc.vector.reciprocal(rcp[:], tp1[:, 64:65])
                    nc.vector.tensor_scalar_mul(
                        ctxn[:, j * 64:(j + 1) * 64], tp1[:, 0:64], rcp[:])
                tp2 = ps3b.tile([128, 128], BF16, tag="tp2", name=f"tp2_{t}_{p}")
                nc.tensor.transpose(tp2[:], ctxn[:], idb[:])
                lt = ltp.tile([128, 128], BF16, tag="lt", name=f"lt{t}_{p}")
                nc.scalar.copy(lt[:], tp2[:])
                lts.append(lt)
            ot = osbp.tile([128, D], F32, tag="ot", name=f"ot{t}")
            for n2 in range(2):
                po = pop.tile([128, 512], F32, tag="po", name=f"po{t}_{n2}")
                for p in range(NP):
                    nc.tensor.matmul(
                        po[:], lts[p][:],
                        wo_sb[:, p * D + n2 * 512: p * D + (n2 + 1) * 512],
                        start=(p == 0), stop=(p == NP - 1))
                if n2 == 0:
                    nc.scalar.copy(ot[:, 0:512], po[:])
                else:
                    nc.vector.tensor_copy(ot[:, 512:1024], po[:])
            nc.sync.dma_start(out[ts_, :], ot[:])

_CACHE = {}


def _build():
    if "nc" in _CACHE:
        return _CACHE["nc"]
    nc = bacc.Bacc("TRN2", target_bir_lowering=False, debug=False)
    ins = {
        "X": nc.dram_tensor("X", [S, D], F32, kind="ExternalInput").ap(),
        "Wq": nc.dram_tensor("Wq", [D, DC], F32, kind="ExternalInput").ap(),
        "bq": nc.dram_tensor("bq", [DC], F32, kind="ExternalInput").ap(),
        "Wk": nc.dram_tensor("Wk", [D, DC], F32, kind="ExternalInput").ap(),
        "bk": nc.dram_tensor("bk", [DC], F32, kind="ExternalInput").ap(),
        "Wv": nc.dram_tensor("Wv", [D, DC], F32, kind="ExternalInput").ap(),
        "bv": nc.dram_tensor("bv", [DC], F32, kind="ExternalInput").ap(),
        "Wo": nc.dram_tensor("Wo", [DC, D], F32, kind="ExternalInput").ap(),
    }
    outp = nc.dram_tensor("out", [S, D], F32, kind="ExternalOutput").ap()
    with tile.TileContext(nc) as tcx:
        with ExitStack() as ctx:
            _emit(ctx, tcx, ins, outp)
    nc.compile()
    _CACHE["nc"] = nc
    return nc


def core_inputs(X, Wq, bq, Wk, bk, Wv, bv, Wo, core):
    b, g = divmod(core, 4)
    cs = slice(g * DC, (g + 1) * DC)
    return {
        "X": np.ascontiguousarray(X[b]),
        "Wq": np.ascontiguousarray(Wq[:, cs]), "bq": np.ascontiguousarray(bq[cs]),
        "Wk": np.ascontiguousarray(Wk[:, cs]), "bk": np.ascontiguousarray(bk[cs]),
        "Wv": np.ascontiguousarray(Wv[:, cs]), "bv": np.ascontiguousarray(bv[cs]),
        "Wo": np.ascontiguousarray(Wo[cs, :]),
    }


def kernel(X, Wq, bq, Wk, bk, Wv, bv, Wo, bo, _trace=False):
    nc = _build()
    in_maps = [core_inputs(X, Wq, bq, Wk, bk, Wv, bv, Wo, c) for c in range(8)]
    res = run_bass_kernel_spmd(nc, in_maps, list(range(8)), trace=_trace)
    parts = [res.results[c]["out"] for c in range(8)]
    full = np.stack([
        parts[0] + parts[1] + parts[2] + parts[3] + bo,
        parts[4] + parts[5] + parts[6] + parts[7] + bo,
    ]).astype(np.float32)
    if _trace:
        return full, res
    return full


# revision 36
# speedup vs baseline: 1.1903x; 1.1903x over previous
"""Multi-head attention (B=2, S=4096, D=1024, H=16, HD=64) on 8 trn2 cores.

Sharding: core c -> batch b = c//4, head-group g = c%4 (4 heads per core).
Each core: Q/K/V projections for its heads on its batch, attention, and the
partial output ctx @ Wo[rows of its heads]. Host sums the 4 partials per
batch and adds bo.

Design notes (measured on HW, perfetto/NTFF traces):
  - The whole kernel is ActE(exp)/PE co-limited in phase 2; everything
    else is engine-balancing around those two walls.
  - All matmul operands bf16 (fp8 fails the 2e-2 gate: measured 4.1e-2
    end-to-end; bf16 measures ~7e-3). PSUM stays fp32.
  - exp: 5/8 of score tiles on the scalar engine (Exp activation,
    PSUM->SBUF bf16, 1/8 scale folded, no max-subtraction since scores
    are O(6)); 3/8 on the vector engine via a one-op int16-domain
    Schraudolph (bf16_bits = int16(x*2^7/ln2 + (127<<7) - 7), bitcast
    int16->bf16 is free). Softmax normalization cancels most of the
    Schraudolph error.
  - X^T via PE transposes in bf16 (cast on ActE first); 4 transposes
    batched per PSUM tile to amortize the DVE copy-out.
  - Q^T/K^T [dq, tok] bf16 with head-pairs packed on partition halves:
    scores = two 64-row PE tiles that the HW co-schedules (row groups).
    PV likewise runs as 64-row tile pairs into 4 separate accumulators —
    merging the k-halves into one accumulator (interleaved accumulation
    groups on one bank) crashes the exec unit, hence the final add.
  - V token-major bf16 with a ones column per head (V'[k, 65]); PV
    accumulates ctx^T and the softmax denominator together.
  - ctx'' stays resident in SBUF (bf16); phase 3 (normalize + Wo) is
    DMA-free until the output store.
  - PSUM is the scarce resource: phase 2 uses all 8 banks (2x2 score
    tiles + 4 accumulators), which is what prevents phase overlap.
"""

import os
from contextlib import ExitStack

import numpy as np

os.environ.setdefault("MYCRO_LOCAL_CACHE", "1")

import concourse.bass as bass
import concourse.tile as tile
from concourse import bacc, mybir
from concourse.bass_utils import run_bass_kernel_spmd
from concourse.masks import make_identity

F32 = mybir.dt.float32
F32R = mybir.dt.float32r
BF16 = mybir.dt.bfloat16
I16 = mybir.dt.int16
AF = mybir.ActivationFunctionType

S = 4096          # sequence length
D = 1024          # model dim
HC = 4            # heads per core
HD = 64           # head dim
DC = HC * HD      # 256 per-core projection width
NP = HC // 2      # head pairs per core
KT = S // 128     # 32 k-tiles
QC = S // 512     # 8 q-chunks of 512
TC = S // 512     # 8 tok-chunks of 512
SCALE = 1.0 / 8.0

# Schraudolph-exp constants (DVE offload), int16/bf16-bit domain:
# bf16_bits(exp(x)) ~ int16(x * 2^7/ln2 + (127<<7) - C), one DVE op with a
# free bitcast int16->bf16. The scores 1/8 scale is folded into A. The
# scalar engine saturates on exp; routing 3/8 of the tiles to the idle
# vector engine costs ~3% bounded relative error on those attention
# weights, washed out by the softmax normalization (end-to-end measured
# 6.5e-3 vs the 2e-2 gate).
EXP_A = float((1 << 7) / np.log(2.0) * SCALE)
EXP_B = float((127 << 7) - 7)
SCHRAUD_K = frozenset(k for k in range(KT) if k % 8 in (1, 3, 6))


def _emit(ctx: ExitStack, tc: tile.TileContext, ins: dict, out: bass.AP):
    nc = tc.nc
    X, Wq, bq, Wk, bk, Wv, bv, Wo = (
        ins["X"], ins["Wq"], ins["bq"], ins["Wk"], ins["bk"], ins["Wv"],
        ins["bv"], ins["Wo"],
    )

    const = ctx.enter_context(tc.tile_pool(name="const", bufs=1))
    ident = const.tile([128, 128], F32)
    make_identity(nc, ident[:])
    idb = const.tile([128, 128], BF16, tag="idb")
    nc.vector.tensor_copy(idb[:], ident[:])

    # All weights bf16 (X is cast to bf16 before the transpose/projections).
    wq_sb = const.tile([128, 8 * DC], BF16, tag="wq")
    wk_sb = const.tile([128, 8 * DC], BF16, tag="wk")
    wv_sb = const.tile([128, 8 * DC], BF16, tag="wv")
    wo_sb = const.tile([128, 2 * D], BF16, tag="wo")
    with tc.tile_pool(name="wstage", bufs=2) as wst:
        for dst, src, nchunks in ((wq_sb, Wq, 8), (wk_sb, Wk, 8),
                                  (wv_sb, Wv, 8), (wo_sb, Wo, 2)):
            stg = wst.tile([128, dst.shape[1]], F32, tag="wstg",
                           name=f"wstg_{src.name}")
            nc.gpsimd.dma_start(stg[:].rearrange("p (c d) -> p c d", c=nchunks),
                                src.rearrange("(c p) d -> p c d", p=128))
            nc.vector.tensor_copy(dst[:], stg[:])
    bq_sb = const.tile([128, 2], F32, tag="bq")
    bk_sb = const.tile([128, 2], F32, tag="bk")
    nc.gpsimd.dma_start(bq_sb[:], bq.rearrange("(c p) -> p c", p=128))
    nc.gpsimd.dma_start(bk_sb[:], bk.rearrange("(c p) -> p c", p=128))
    bv_bc = const.tile([128, DC], F32, tag="bv")
    nc.gpsimd.dma_start(bv_bc[:], bv.unsqueeze(0).to_broadcast([128, DC]))
    ones4 = const.tile([128, HC], F32, tag="ones4")
    nc.vector.memset(ones4[:], 1.0)

    # Activations live for the whole kernel (all bf16 now; ctx in SBUF).
    acts = ctx.enter_context(tc.tile_pool(name="acts", bufs=1))
    QT = [acts.tile([128, S], BF16, tag=f"qt{p}", name=f"qt{p}") for p in range(NP)]
    KT_ = [acts.tile([128, S], BF16, tag=f"kt{p}", name=f"ktile{p}") for p in range(NP)]
    VPA = acts.tile([128, KT, HC * 65], BF16, tag="vpa", name="vpa")
    VP = [VPA[:, k, :] for k in range(KT)]
    ctx_sb = acts.tile([65, HC, S], BF16, tag="ctxs", name="ctxs")

    # ---------------- Phase 1: X^T + projections ----------------
    with nc.named_scope("ph1"), \
         tc.tile_pool(name="xrow", bufs=5) as xrow, \
         tc.tile_pool(name="xb", bufs=5) as xbp, \
         tc.tile_pool(name="xt", bufs=16) as xtp, \
         tc.tile_pool(name="ps1", bufs=4, space="PSUM") as ps1, \
         tc.tile_pool(name="ps2", bufs=2, space="PSUM") as ps2:
        for t8 in range(TC):
            xts = [xrow.tile([128, D], F32, tag="xr", name=f"xr{t8}_{i}") for i in range(4)]
            xbs = [xbp.tile([128, D], BF16, tag="xb", name=f"xb{t8}_{i}") for i in range(4)]
            for tt in range(4):
                nc.sync.dma_start(xts[tt][:], X[t8 * 512 + tt * 128:
                                              t8 * 512 + (tt + 1) * 128, :])
                nc.scalar.copy(xbs[tt][:], xts[tt][:])
            xt = [xtp.tile([128, 512], BF16, tag="xt", name=f"xt{t8}_{i}") for i in range(8)]
            for dc in range(8):
                pt = ps1.tile([128, 4, 128], BF16, tag="tp")
                for tt in range(4):
                    nc.tensor.transpose(
                        pt[:, tt, :], xbs[tt][:, dc * 128:(dc + 1) * 128],
                        idb[:])
                nc.vector.tensor_copy(xt[dc][:], pt[:].rearrange("p a b -> p (a b)"))
            for p in range(NP):
                pq = ps2.tile([128, 512], F32, tag="pq")
                for dc in range(8):
                    nc.tensor.matmul(
                        pq[:], wq_sb[:, dc * DC + p * 128: dc * DC + (p + 1) * 128],
                        xt[dc][:], start=(dc == 0), stop=(dc == 7))
                nc.scalar.activation(
                    QT[p][:, t8 * 512:(t8 + 1) * 512], pq[:], AF.Identity,
                    bias=bq_sb[:, p:p + 1])
                pk = ps2.tile([128, 512], F32, tag="pq")
                for dc in range(8):
                    nc.tensor.matmul(
                        pk[:], wk_sb[:, dc * DC + p * 128: dc * DC + (p + 1) * 128],
                        xt[dc][:], start=(dc == 0), stop=(dc == 7))
                nc.vector.tensor_scalar_add(
                    KT_[p][:, t8 * 512:(t8 + 1) * 512], pk[:], bk_sb[:, p:p + 1])
            for tt in range(4):
                kt = t8 * 4 + tt
                pv = ps2.tile([128, 256], F32, tag="pv")
                for dc in range(8):
                    nc.tensor.matmul(
                        pv[:], xt[dc][:, tt * 128:(tt + 1) * 128],
                        wv_sb[:, dc * DC:(dc + 1) * DC],
                        start=(dc == 0), stop=(dc == 7))
                vdst = VP[kt][:].rearrange("p (h w) -> p h w", h=HC)[:, :, 0:64]
                nc.vector.scalar_tensor_tensor(
                    vdst, pv[:].rearrange("p (h w) -> p h w", h=HC), 1.0,
                    bv_bc[:].rearrange("p (h w) -> p h w", h=HC),
                    mybir.AluOpType.bypass, mybir.AluOpType.add)
                ones = VP[kt][:].rearrange("p (h w) -> p h w", h=HC)[:, :, 64:65]
                nc.vector.tensor_copy(ones, ones4[:].unsqueeze(2))

    # ---------------- Phase 2: attention ----------------
    with nc.named_scope("ph2"), \
         tc.tile_pool(name="sps", bufs=2, space="PSUM") as sps, \
         tc.tile_pool(name="pvs", bufs=4, space="PSUM") as pvs, \
         tc.tile_pool(name="et", bufs=8) as etp, \
         tc.tile_pool(name="eti", bufs=6) as etip:
        seq = [(p, qc, k) for p in range(NP) for qc in range(QC)
               for k in range(KT)]
        accs = {}
        ets = {}

        def s_step(i):
            p, qc, k = seq[i]
            qs = slice(qc * 512, (qc + 1) * 512)
            ks = slice(k * 128, (k + 1) * 128)
            st = sps.tile([128, 1024], F32, tag="st", name=f"st{p}_{qc}_{k}")
            nc.tensor.matmul(st[:, 0:512], KT_[p][0:64, ks],
                             QT[p][0:64, qs], start=True, stop=True)
            nc.tensor.matmul(st[:, 512:1024], KT_[p][64:128, ks],
                             QT[p][64:128, qs], start=True, stop=True)
            if k in SCHRAUD_K:
                ei = etip.tile([128, 1024], I16, tag="eti",
                               name=f"eti{p}_{qc}_{k}")
                nc.vector.tensor_scalar(ei[:], st[:], EXP_A, EXP_B,
                                        mybir.AluOpType.mult,
                                        mybir.AluOpType.add)
                et = ei[:].bitcast(BF16)
            else:
                ett = etp.tile([128, 1024], BF16, tag="et",
                               name=f"et{p}_{qc}_{k}")
                nc.scalar.activation(ett[:], st[:], AF.Exp, bias=0.0,
                                     scale=SCALE)
                et = ett[:]
            ets[i] = et

        LOOKAHEAD = 2
        for i in range(LOOKAHEAD):
            s_step(i)
        for i, (p, qc, k) in enumerate(seq):
            if i + LOOKAHEAD < len(seq):
                s_step(i + LOOKAHEAD)
            if k == 0:
                accs[(p, qc)] = [
                    pvs.tile([65, 512], F32, tag="acc", name=f"acc{p}_{qc}_{j2}")
                    for j2 in range(4)]
            acc = accs[(p, qc)]
            et = ets.pop(i)
            for j in range(2):
                h = 2 * p + j
                vs = slice(h * 65, (h + 1) * 65)
                es = slice(j * 512, (j + 1) * 512)
                nc.tensor.matmul(
                    acc[2 * j][:], VP[k][0:64, vs], et[0:64, es],
                    start=(k == 0), stop=(k == KT - 1),
                    skip_group_check=True)
                nc.tensor.matmul(
                    acc[2 * j + 1][:], VP[k][64:128, vs],
                    et[64:128, es],
                    start=(k == 0), stop=(k == KT - 1),
                    skip_group_check=True)
            if k == KT - 1:
                qs = slice(qc * 512, (qc + 1) * 512)
                for j in range(2):
                    h = 2 * p + j
                    btmp = etp.tile([65, 512], F32, tag="btmp",
                                    name=f"btmp{p}_{qc}_{j}")
                    nc.scalar.copy(btmp[:], acc[2 * j + 1][:])
                    nc.vector.tensor_add(ctx_sb[:, h, qs], acc[2 * j][:],
                                         btmp[:])
                del accs[(p, qc)]

    # ---------------- Phase 3: normalize + Wo ----------------
    with nc.named_scope("ph3"), \
         tc.tile_pool(name="ps3a", bufs=4, space="PSUM") as ps3a, \
         tc.tile_pool(name="ps3b", bufs=2, space="PSUM") as ps3b, \
         tc.tile_pool(name="po", bufs=2, space="PSUM") as pop, \
         tc.tile_pool(name="ctxn", bufs=3) as ctxnp, \
         tc.tile_pool(name="rcpp", bufs=8) as rcpp, \
         tc.tile_pool(name="ltp", bufs=4) as ltp, \
         tc.tile_pool(name="osb", bufs=4) as osbp:
        for t in range(S // 128):
            ts_ = slice(t * 128, (t + 1) * 128)
            lts = []
            for p in range(NP):
                ctxn = ctxnp.tile([128, 128], BF16, tag="ctxn",
                                  name=f"ctxn{t}_{p}")
                for j in range(2):
                    h = 2 * p + j
                    tp1 = ps3a.tile([128, 65], BF16, tag="tp1",
                                    name=f"tp1_{t}_{h}")
                    nc.tensor.transpose(tp1[:], ctx_sb[:, h, ts_],
                                        idb[0:65, 0:65])
                    rcp = rcpp.tile([128, 1], F32, tag="rcp",
                                    name=f"rcp{t}_{h}")
                    nc.vector.reciprocal(rcp[:], tp1[:, 64:65])
                    nc.vector.tensor_scalar_mul(
                        ctxn[:, j * 64:(j + 1) * 64], tp1[:, 0:64], rcp[:])
                tp2 = ps3b.tile([128, 128], BF16, tag="tp2", name=f"tp2_{t}_{p}")
                nc.tensor.transpose(tp2[:], ctxn[:], idb[:])
                lt = ltp.tile([128, 128], BF16, tag="lt", name=f"lt{t}_{p}")
                nc.scalar.copy(lt[:], tp2[:])
                lts.append(lt)
            ot = osbp.tile([128, D], F32, tag="ot", name=f"ot{t}")
            for n2 in range(2):
                po = pop.tile([128, 512], F32, tag="po", name=f"po{t}_{n2}")
                for p in range(NP):
                    nc.tensor.matmul(
                        po[:], lts[p][:],
                        wo_sb[:, p * D + n2 * 512: p * D + (n2 + 1) * 512],
                        start=(p == 0), stop=(p == NP - 1))
                if n2 == 0:
                    nc.scalar.copy(ot[:, 0:512], po[:])
                else:
                    nc.vector.tensor_copy(ot[:, 512:1024], po[:])
            nc.sync.dma_start(out[ts_, :], ot[:])

_CACHE = {}


def _build():
    if "nc" in _CACHE:
        return _CACHE["nc"]
    nc = bacc.Bacc("TRN2", target_bir_lowering=False, debug=False)
    ins = {
        "X": nc.dram_tensor("X", [S, D], F32, kind="ExternalInput").ap(),
        "Wq": nc.dram_tensor("Wq", [D, DC], F32, kind="ExternalInput").ap(),
        "bq": nc.dram_tensor("bq", [DC], F32, kind="ExternalInput").ap(),
        "Wk": nc.dram_tensor("Wk", [D, DC], F32, kind="ExternalInput").ap(),
        "bk": nc.dram_tensor("bk", [DC], F32, kind="ExternalInput").ap(),
        "Wv": nc.dram_tensor("Wv", [D, DC], F32, kind="ExternalInput").ap(),
        "bv": nc.dram_tensor("bv", [DC], F32, kind="ExternalInput").ap(),
        "Wo": nc.dram_tensor("Wo", [DC, D], F32, kind="ExternalInput").ap(),
    }
    outp = nc.dram_tensor("out", [S, D], F32, kind="ExternalOutput").ap()
    with tile.TileContext(nc) as tcx:
        with ExitStack() as ctx:
            _emit(ctx, tcx, ins, outp)
    nc.compile()
    _CACHE["nc"] = nc
    return nc


def core_inputs(X, Wq, bq, Wk, bk, Wv, bv, Wo, core):
    b, g = divmod(core, 4)
    cs = slice(g * DC, (g + 1) * DC)
    return {
        "X": np.ascontiguousarray(X[b]),
        "Wq": np.ascontiguousarray(Wq[:, cs]), "bq": np.ascontiguousarray(bq[cs]),
        "Wk": np.ascontiguousarray(Wk[:, cs]), "bk": np.ascontiguousarray(bk[cs]),
        "Wv": np.ascontiguousarray(Wv[:, cs]), "bv": np.ascontiguousarray(bv[cs]),
        "Wo": np.ascontiguousarray(Wo[cs, :]),
    }


def kernel(X, Wq, bq, Wk, bk, Wv, bv, Wo, bo, _trace=False):
    nc = _build()
    in_maps = [core_inputs(X, Wq, bq, Wk, bk, Wv, bv, Wo, c) for c in range(8)]
    res = run_bass_kernel_spmd(nc, in_maps, list(range(8)), trace=_trace)
    parts = [res.results[c]["out"] for c in range(8)]
    full = np.stack([
        parts[0] + parts[1] + parts[2] + parts[3] + bo,
        parts[4] + parts[5] + parts[6] + parts[7] + bo,
    ]).astype(np.float32)
    if _trace:
        return full, res
    return full
